# revision 16
# baseline (speedup 1.0000x reference)
"""Trainium2 Bass kernel for nn_DefuzzyLayer: out = x @ rules_outcome.

x: [8192, 4096] f32, rules_outcome: [4096, 4096] f32 -> out: [8192, 4096] f32.

Strategy: data-parallel over batch. Each of the 8 NeuronCores computes a
[1024, 4096] output shard: x_shard @ W with the full W replicated.

Per-core kernel (Tile framework), fp8 DoubleRow path (default):
  - Host quantizes x and (W - 0.5) to fp8 e4m3 (TRN FP8_EXP4 == IEEE e4m3).
    Mean-centering W halves its quantization error; the exact correction
    0.5*rowsum(x) is added on the host after the kernel (rowsum is 0.01%
    of the FLOPs).  Net rel-err ~1.8e-2.
  - DoubleRow perf mode: both matmul operands are 3D APs [128, 2, free];
    the PE virtualizes to 128x256, contracting 256 per instruction at
    2 fp8 MACs/cell/cycle.
  - W is the stationary operand, x the moving one: each [128, 2, 128]
    w-stationary serves two [128, 2, 512] x-moving matmuls (the two halves
    of the 1024-row batch shard), so the 256-column LDWEIGHTS (the HW
    bottleneck: not overlapped in x-stationary form, measured 272 ns/MM)
    is amortized over ~214 ns of PE streaming.  PSUM comes out [n, m];
    the output is stored transposed and the host transposes it back.
  - Blocks 2+ process their four 128-wide n-subblocks as two passes of
    two, so PSUM eviction of one pass overlaps matmuls of the other.
    Blocks 0-1 run single-pass (their operands are still streaming in;
    one pass halves the x/w chunk consumption rate).
  - DMA: w loads on the SP ring (blocks 0/1 split into sub-loads so the
    first matmul starts after ~0.5 MiB arrives), x on the ACT ring,
    stores alternate rings.  Eviction converts to fp16 (store traffic
    halves; adds ~5e-4 relative error); host casts back to fp32.
"""

import os

import numpy as np

BATCH = 8192
IN_DIM = 4096
OUT_DIM = 4096
N_CORES = 8
M_SHARD = BATCH // N_CORES  # 1024

P = 128
KT = IN_DIM // P            # 32 k-tiles
MT = M_SHARD // P           # 8 m-tiles
NSUB = 4                    # 128-wide n-subblocks per n-block
MHALVES = 2                 # moving m-halves (512 each)
MW = M_SHARD // MHALVES     # 512

IN_DT = os.environ.get("KDT", "float8e4")  # float8e4 | float16 | bfloat16
NB = int(os.environ.get("KNB", "512" if IN_DT == "float8e4" else "256"))
NBLK = OUT_DIM // NB        # n-blocks
XCHUNKS = int(os.environ.get("KXC", "8"))  # x load split (fp16 path)
KPC = KT // XCHUNKS         # k-tiles per x chunk (fp16 path)
PS_BUFS = int(os.environ.get("KPSBUFS", "8"))
WBUFS = int(os.environ.get("KWB", "3"))
OUT_DT = os.environ.get("KODT", "float16")     # on-device output dtype


def _sizes(env, default):
    v = os.environ.get(env)
    s = [int(t) for t in v.split(",")] if v else default
    assert sum(s) == KT
    return s


# geometric k-tile chunking: small first loads for fast startup, large later
# ones to bound DMA count (descriptor-gen serializes at ~0.6us per DMA)
XSIZES = _sizes("KXS", [2, 2, 4, 4, 4, 4, 4, 4, 4])
W0SIZES = _sizes("KW0S", [2, 2, 4, 8, 8, 8])
W1SIZES = _sizes("KW1S", [4, 4, 8, 8, 8])
FULLM_BLOCKS = int(os.environ.get("KFULLM", "2"))  # leading single-pass blocks

_cached_nc = None


def _np_dt():
    import ml_dtypes
    if IN_DT == "float16":
        return np.dtype(np.float16)
    if IN_DT == "bfloat16":
        return np.dtype(ml_dtypes.bfloat16)
    if IN_DT == "float8e4":
        return np.dtype(ml_dtypes.float8_e4m3)
    return np.dtype(np.float32)


def _build(loop_n=1, in_dt=None, variant="full"):
    """Build + compile the per-core Bass module.

    loop_n > 1 wraps the whole body in an on-device For_i loop -- used only
    for HW timing (amortizes host dispatch overhead out of the measurement).
    variant: "full" | "nodma" (skip x/w loads) | "dmaonly" (skip compute)
             | "mmonly" (skip loads + eviction).
    """
    import contextlib
    import concourse.bacc as bacc
    import concourse.tile as tile
    import concourse.mybir as mybir

    do_in_dma = variant not in ("nodma", "mmonly")
    do_compute = variant != "dmaonly"
    do_evict = variant != "mmonly"

    in_dt = in_dt or IN_DT
    fp8 = in_dt == "float8e4"
    dt_in = getattr(mybir.dt, in_dt)
    dt_out = getattr(mybir.dt, OUT_DT)

    nc = bacc.Bacc("TRN2", target_bir_lowering=False, debug=False)
    # partition-major packed inputs (see _pack_x_shard/_pack_w)
    xt = nc.dram_tensor(
        "xt", [P, KT, M_SHARD], dt_in, kind="ExternalInput"
    ).ap()
    w = nc.dram_tensor(
        "w", [P, NBLK * KT, NB], dt_in, kind="ExternalInput"
    ).ap()
    if fp8:
        # transposed output [n, m]; host transposes back
        out = nc.dram_tensor(
            "out", [OUT_DIM, M_SHARD], dt_out, kind="ExternalOutput"
        ).ap()
        out_r = out.rearrange("(t p) m -> p t m", p=P)  # [128, 32, M_SHARD]
    else:
        out = nc.dram_tensor(
            "out", [M_SHARD, OUT_DIM], dt_out, kind="ExternalOutput"
        ).ap()
        out_r = out.rearrange("(m p) n -> p m n", p=P)  # [128, MT, OUT_DIM]

    DR = mybir.MatmulPerfMode.DoubleRow

    with tile.TileContext(nc) as tc:
        loop_ctx = (
            tc.For_i(0, loop_n, 1,
                     hint_engines=(mybir.EngineType.PE, mybir.EngineType.SP,
                                   mybir.EngineType.DVE))
            if loop_n > 1 else contextlib.nullcontext()
        )
        with (
            loop_ctx,
            tc.tile_pool(name="xpool", bufs=len(XSIZES) if fp8 else XCHUNKS)
            as xpool,
            tc.tile_pool(name="w0pool", bufs=len(W0SIZES)) as w0pool,
            tc.tile_pool(name="w1pool", bufs=len(W1SIZES)) as w1pool,
            tc.tile_pool(name="wpool", bufs=WBUFS) as wpool,
            tc.tile_pool(name="opool", bufs=8) as opool,
            tc.tile_pool(name="pspool", bufs=PS_BUFS, space="PSUM") as pspool,
        ):
            def load_chunks(pool, sizes, width, dram_ap, nm):
                """Allocate [P, size, width] tiles and DMA each k-tile range
                on the SP (w) / ACT (x) ring; return (tiles, kt->tile map)."""
                tiles, ktmap, k0 = [], {}, 0
                for ci, sz in enumerate(sizes):
                    tl = pool.tile([P, sz, width], dt_in,
                                   name=f"{nm}{ci}", tag=nm)
                    if do_in_dma:
                        eng = nc.scalar if nm == "x" else nc.sync
                        eng.dma_start(out=tl[:],
                                      in_=dram_ap[:, k0:k0 + sz, :])
                    else:
                        nc.vector.memset(tl[:, 0, 0:1], 0.0)
                    for r in range(sz):
                        ktmap[k0 + r] = (tl, r)
                    tiles.append(tl)
                    k0 += sz
                return tiles, ktmap

            x_chunks = None
            if fp8:
                # interleave x / w0 issue so small startup pieces of both
                # land first in the global DMA queue
                _, w0map = load_chunks(
                    w0pool, W0SIZES, NB, w[:, 0:KT, :], "w0")
                _, xmap = load_chunks(xpool, XSIZES, M_SHARD, xt, "x")
                _, w1map = load_chunks(
                    w1pool, W1SIZES, NB, w[:, KT:2 * KT, :], "w1")
            else:
                x_chunks = []
                for c in range(XCHUNKS):
                    x_c = xpool.tile([P, KPC, M_SHARD], dt_in,
                                     name=f"x{c}", tag="x")
                    if do_in_dma:
                        nc.scalar.dma_start(
                            out=x_c[:],
                            in_=xt[:, c * KPC:(c + 1) * KPC, :],
                        )
                    else:
                        nc.vector.memset(x_c[:, 0, 0:1], 0.0)
                    x_chunks.append(x_c)

            shared_psums = None
            if fp8 and not do_evict:
                shared_psums = {
                    (s, h): pspool.tile([P, MW], mybir.dt.float32,
                                        name=f"sps{s}_{h}", tag="ps")
                    for s in range(NSUB) for h in range(MHALVES)
                }

            def w_slice(b, w_b, kk, s):
                j0 = s * P
                if fp8 and b <= 1:
                    tl, r = (w0map if b == 0 else w1map)[kk]
                    return tl[:, r:r + 2, j0:j0 + P]
                return w_b[:, kk:kk + 2, j0:j0 + P]

            for b in range(NBLK):
                w_b = None
                if not (fp8 and b <= 1):
                    w_b = wpool.tile([P, KT, NB], dt_in, name=f"w{b}", tag="w")
                    if do_in_dma:
                        nc.sync.dma_start(
                            out=w_b[:],
                            in_=w[:, b * KT:(b + 1) * KT, :],
                        )
                    else:
                        nc.vector.memset(w_b[:, 0, 0:1], 0.0)

                if not do_compute:
                    continue
                if fp8:
                    # n-subblock passes: leading blocks single-pass (operand
                    # streaming), later ones two passes so eviction overlaps
                    passes = ([list(range(NSUB))] if b < FULLM_BLOCKS
                              else [[0, 1], [2, 3]])
                    for subs in passes:
                        if shared_psums is not None:
                            psums = {k: shared_psums[k]
                                     for k in shared_psums if k[0] in subs}
                        else:
                            psums = {
                                (s, h): pspool.tile(
                                    [P, MW], mybir.dt.float32,
                                    name=f"ps{b}_{s}_{h}", tag="ps")
                                for s in subs for h in range(MHALVES)
                            }
                        for kk in range(0, KT, 2):
                            xc, ko = xmap[kk]
                            for s in subs:
                                ws = w_slice(b, w_b, kk, s)
                                for h in range(MHALVES):
                                    nc.tensor.matmul(
                                        psums[(s, h)][:],
                                        ws,
                                        xc[:, ko:ko + 2, h * MW:(h + 1) * MW],
                                        start=(kk == 0),
                                        stop=(kk == KT - 2),
                                        perf_mode=DR,
                                    )
                        if do_evict:
                            for s in subs:
                                # o_s: [P, 2, MW] fp16 = one 128-row slice of
                                # the transposed output, both m-halves
                                o_s = opool.tile([P, MHALVES, MW], dt_out,
                                                 name=f"o{b}_{s}", tag="o")
                                for h in range(MHALVES):
                                    if (s + h) % 2 == 0:
                                        nc.vector.tensor_copy(
                                            o_s[:, h, :], psums[(s, h)][:])
                                    else:
                                        nc.scalar.copy(
                                            o_s[:, h, :], psums[(s, h)][:])
                                eng = nc.sync if s % 2 == 0 else nc.scalar
                                eng.dma_start(
                                    out=out_r[:, b * NSUB + s, :],
                                    in_=o_s[:],
                                )
                else:
                    o_b = None
                    if do_evict:
                        o_b = opool.tile([P, MT, NB], dt_out,
                                         name=f"o{b}", tag="o")
                    if shared_psums is not None:
                        psums = shared_psums
                    else:
                        psums = [
                            pspool.tile([P, NB], mybir.dt.float32,
                                        name=f"ps{b}_{m}", tag="ps")
                            for m in range(MT)
                        ]
                    for k in range(KT):
                        xc = x_chunks[k // KPC]
                        ko = k % KPC
                        for m in range(MT):
                            nc.tensor.matmul(
                                psums[m][:],
                                xc[:, ko, m * P:(m + 1) * P],
                                w_b[:, k, :],
                                start=(k == 0),
                                stop=(k == KT - 1),
                            )
                    if do_evict:
                        for m in range(MT):
                            nc.vector.tensor_copy(o_b[:, m, :], psums[m][:])
                        nc.scalar.dma_start(
                            out=out_r[:, :, b * NB:(b + 1) * NB],
                            in_=o_b[:],
                        )

    nc.compile()
    return nc


def _get_nc():
    global _cached_nc
    if _cached_nc is None:
        _cached_nc = _build()
    return _cached_nc


def _pack_x_shard(x_shard_q):
    """[M_SHARD, IN_DIM] -> [128, KT, M_SHARD] partition-major."""
    # dest[p, k, m] = x_shard[m, k*128 + p]
    return np.ascontiguousarray(
        x_shard_q.T.reshape(KT, P, M_SHARD).transpose(1, 0, 2)
    )


def _pack_w(w_q):
    """[IN_DIM, OUT_DIM] -> [128, NBLK*KT, NB] partition-major."""
    # dest[p, b*KT + k, j] = w[k*128 + p, b*NB + j]
    return np.ascontiguousarray(
        w_q.reshape(KT, P, NBLK, NB).transpose(1, 2, 0, 3).reshape(P, NBLK * KT, NB)
    )


def _make_in_maps(x, rules_outcome):
    np_dt = _np_dt()
    fp8 = IN_DT == "float8e4"
    x = np.asarray(x, dtype=np.float32)
    w = np.asarray(rules_outcome, dtype=np.float32)
    assert x.shape == (BATCH, IN_DIM) and w.shape == (IN_DIM, OUT_DIM)
    if fp8:
        w_packed = _pack_w((w - np.float32(0.5)).astype(np_dt))
    else:
        w_packed = _pack_w(w.astype(np_dt))
    return [
        {
            "xt": _pack_x_shard(
                x[i * M_SHARD:(i + 1) * M_SHARD, :].astype(np_dt)),
            "w": w_packed,
        }
        for i in range(N_CORES)
    ]


def _run(x, rules_outcome, **spmd_kwargs):
    from concourse.bass_utils import run_bass_kernel_spmd

    fp8 = IN_DT == "float8e4"
    x = np.asarray(x, dtype=np.float32)
    in_maps = _make_in_maps(x, rules_outcome)
    nc = _get_nc()
    res = run_bass_kernel_spmd(nc, in_maps, core_ids=list(range(N_CORES)),
                               **spmd_kwargs)
    if fp8:
        shards = [np.asarray(res.results[i]["out"]).T.astype(np.float32)
                  for i in range(N_CORES)]
        full = np.concatenate(shards, axis=0)
        # exact mean-centering correction: out += 0.5 * rowsum(x)
        full += 0.5 * x.sum(axis=1, dtype=np.float64).astype(
            np.float32)[:, None]
    else:
        full = np.concatenate(
            [np.asarray(res.results[i]["out"], dtype=np.float32)
             for i in range(N_CORES)], axis=0)
    return full, res


def kernel(x, rules_outcome):
    out, _ = _run(x, rules_outcome)
    return out
